# revision 11
# baseline (speedup 1.0000x reference)
"""DCNv2 (modulated deformable conv 3x3) for Trainium2, 8 NeuronCores.

Sharding: pure data-parallel over batch B=8 -> core b computes batch b.

Per-core algorithm (batch b, C=Cout=128, H=W=64, P=H*W=4096):
  1. PE (fp32): offset/mask conv as 9 accumulated matmuls over a zero-padded
     channel-major x ([128, 66*66] SBUF), output [27, P] channel-major.
     ACT applies bias (+ sigmoid for mask rows) during PSUM evacuation.
  2. PE transposes [27,128] chunks -> p-major planes [128(p), 27, 32(pb)].
  3. DVE: bilinear coefficient planes.  floor() via the fp32 round trick
     (x - 0.5 + 1.5*2^23) - 1.5*2^23 (ties resolve either way; bilinear
     interpolation is continuous so both splits give identical samples).
     Per kernel-point k: two gather-row indices (y0c, y1c clipped) packed
     with x_start = clip(x0, 0, 62), and four per-corner coefficients
     C_dl = mask * wy_d * valid_y_d * lane_l  (lane coefs remap the pair
     (x_start, x_start+1) onto bilinear x-corners incl. border clipping).
  4. GPSIMD dma_gather (transpose=True) over a row-major bf16 copy of x
     ([P,128] in DRAM): each int16 index fetches 512B = positions
     (p0, p0+1) x 128 channels, landing transposed as two [c, p] planes.
     2 gathers per k (y-low/y-high rows) = 4 corner planes [128c, 4096p].
  5. PE (bf16): per (k, corner, p-block): Z^T[p,o] = G[c,p-blk].T @ W_k[c,o]
     (gathered block as the stationary operand) -> PSUM [128, 4x128].
  6. ACT evacuates the 4-corner PSUM bank to SBUF; DVE accumulates
     acc[p, o] += coef_corner[p] * Z^T via scalar_tensor_tensor
     (per-partition scalar = per-position coefficient).
  7. Output [P, 128] (p-major) DMAd out; host transposes to [Cout, H, W].
"""

import sys

sys.path.insert(0, "/opt/trn_rl_repo")

import numpy as np
import ml_dtypes

import concourse.bacc as bacc
import concourse.bass as bass
import concourse.mybir as mybir
import concourse.tile as tile
from concourse.ap import AP
from concourse.bass import ts
from concourse.bass_utils import run_bass_kernel_spmd
from concourse.library_config import mlp as mlp_lib
from concourse.masks import make_identity

F32 = mybir.dt.float32
BF16 = mybir.dt.bfloat16
I16 = mybir.dt.int16

B, C, H, W = 8, 128, 64, 64
COUT = 128
K2 = 9
P = H * W            # 4096
NPB = P // 128       # 32 p-blocks
HP = H + 2           # padded side
NROWS = P + 64       # gather table rows (padding for the +1 pair read)
MAGIC = 12582912.0   # 1.5 * 2**23
AOP = mybir.AluOpType
AF = mybir.ActivationFunctionType

_CACHE = {}


def _build(stage="full"):
    import os
    nc = bacc.Bacc("TRN2", target_bir_lowering=False)

    xpad_d = nc.dram_tensor("xpad", [128, HP * HP], F32, kind="ExternalInput")
    xrows_d = nc.dram_tensor("xrows", [NROWS, 128], BF16, kind="ExternalInput")
    wmain_d = nc.dram_tensor("wmain", [K2, 128, COUT], BF16, kind="ExternalInput")
    woff_d = nc.dram_tensor("woff", [K2, 128, 41], F32, kind="ExternalInput")
    bias_d = nc.dram_tensor("bias41", [41, 1], F32, kind="ExternalInput")
    byk_d = nc.dram_tensor("byk", [128, K2, NPB], F32, kind="ExternalInput")
    bxk_d = nc.dram_tensor("bxk", [128, K2, NPB], F32, kind="ExternalInput")
    out_d = nc.dram_tensor("out", [P, COUT], F32, kind="ExternalOutput")

    with tile.TileContext(nc) as tc:
        with (
            tc.tile_pool(name="const", bufs=1) as cp,
            tc.tile_pool(name="coef", bufs=1) as cf,
            tc.tile_pool(name="gp", bufs=4) as gp,
            tc.tile_pool(name="z4p", bufs=4) as z4p,
            tc.tile_pool(name="psO", bufs=2, space="PSUM") as psO,
            tc.tile_pool(name="psT", bufs=2, space="PSUM") as psT,
            tc.tile_pool(name="psZ", bufs=4, space="PSUM") as psZ,
        ):
            nc.gpsimd.load_library(mlp_lib)

            # ---- constant loads ----
            xpad = cp.tile([128, HP * HP], F32)
            nc.gpsimd.dma_start(xpad[:], xpad_d[:])
            wm = cp.tile([128, K2, COUT], BF16)
            nc.gpsimd.dma_start(wm[:], wmain_d[:].rearrange("k c o -> c k o"))
            wo = cp.tile([128, K2, 41], F32)
            nc.gpsimd.dma_start(wo[:], woff_d[:].rearrange("k c j -> c k j"))
            bias = cp.tile([41, 1], F32)
            nc.gpsimd.dma_start(bias[:], bias_d[:])
            byk = cp.tile([128, K2, NPB], F32)
            nc.gpsimd.dma_start(byk[:], byk_d[:])
            bxk = cp.tile([128, K2, NPB], F32)
            nc.gpsimd.dma_start(bxk[:], bxk_d[:])
            ident = cp.tile([64, 64], F32)
            make_identity(nc, ident[:])

            # ---- offset/mask conv: [41, P] channel-major ----
            offs_cm = cf.tile([41, P], F32)
            nc.gpsimd.memset(offs_cm[:], 0.0)
            xv = xpad[:].rearrange("c (h w) -> c h w", h=HP)
            for ch in range(8):
                po = psO.tile([41, 512], F32)
                r0 = ch * 8
                for k in range(K2):
                    ki, kj = k // 3, k % 3
                    rhs = xv[:, r0 + ki : r0 + ki + 8, kj : kj + W]
                    nc.tensor.matmul(
                        po[:], wo[:, k, :], rhs,
                        start=(k == 0), stop=(k == K2 - 1),
                    )
                sl = slice(ch * 512, (ch + 1) * 512)
                nc.scalar.activation(
                    offs_cm[0:18, sl], po[0:18, :], AF.Identity,
                    bias=bias[0:18, :], scale=1.0,
                )
                nc.scalar.activation(
                    offs_cm[32:41, sl], po[32:41, :], AF.Sigmoid,
                    bias=bias[32:41, :], scale=1.0,
                )

            if stage == "conv":
                dbg = nc.dram_tensor("dbg", [41, P], F32, kind="ExternalOutput")
                nc.gpsimd.dma_start(dbg[:], offs_cm[:])
            # ---- transpose to p-major [128, 27, 32] ----
            offs_pm = cf.tile([128, 41, NPB], F32)
            for t in range(NPB):
                pt = psT.tile([128, 41], F32)
                nc.tensor.transpose(
                    pt[:], offs_cm[:, ts(t, 128)], ident[:41, :41]
                )
                nc.vector.tensor_copy(offs_pm[:, :, t], pt[:])

            offy = offs_pm[:, 0:9, :]
            offx = offs_pm[:, 9:18, :]
            mask = offs_pm[:, 32:41, :]

            # ---- coefficient planes (DVE, [128, 9, 32] each) ----
            SH = [128, K2, NPB]

            _tln = [0]

            def tl():
                _tln[0] += 1
                return cf.tile(SH, F32, name=f"cftmp{_tln[0]}")

            def TS(out, in0, s1, op0, s2=None, op1=None):
                kw = {"op1": op1} if op1 is not None else {}
                nc.vector.tensor_scalar(
                    out=out, in0=in0, scalar1=s1, scalar2=s2, op0=op0, **kw
                )

            def TT(out, a, b, op):
                nc.vector.tensor_tensor(out=out, in0=a, in1=b, op=op)

            # y side
            t0 = tl(); TS(t0[:], offy, -0.5, AOP.add, MAGIC, AOP.add)
            iy = tl(); TS(iy[:], t0[:], MAGIC, AOP.subtract)
            fy = tl(); TT(fy[:], offy, iy[:], AOP.subtract)
            ys0 = tl(); TT(ys0[:], iy[:], byk[:], AOP.add)
            ys1 = tl(); TS(ys1[:], ys0[:], 1.0, AOP.add)
            yc0 = tl(); TS(yc0[:], ys0[:], 0.0, AOP.max, 63.0, AOP.min)
            yc1 = tl(); TS(yc1[:], ys1[:], 0.0, AOP.max, 63.0, AOP.min)
            vy0 = tl(); TT(vy0[:], yc0[:], ys0[:], AOP.is_equal)
            vy1 = tl(); TT(vy1[:], yc1[:], ys1[:], AOP.is_equal)
            gy = tl(); TS(gy[:], fy[:], -1.0, AOP.mult, 1.0, AOP.add)
            wy0 = tl(); TT(wy0[:], gy[:], vy0[:], AOP.mult)
            wy1 = tl(); TT(wy1[:], fy[:], vy1[:], AOP.mult)
            my0 = tl(); TT(my0[:], wy0[:], mask, AOP.mult)
            my1 = tl(); TT(my1[:], wy1[:], mask, AOP.mult)
            # x side
            t1 = tl(); TS(t1[:], offx, -0.5, AOP.add, MAGIC, AOP.add)
            ix = tl(); TS(ix[:], t1[:], MAGIC, AOP.subtract)
            fx = tl(); TT(fx[:], offx, ix[:], AOP.subtract)
            xs0 = tl(); TT(xs0[:], ix[:], bxk[:], AOP.add)
            xs1 = tl(); TS(xs1[:], xs0[:], 1.0, AOP.add)
            xc0 = tl(); TS(xc0[:], xs0[:], 0.0, AOP.max, 63.0, AOP.min)
            xc1 = tl(); TS(xc1[:], xs1[:], 0.0, AOP.max, 63.0, AOP.min)
            vx0 = tl(); TT(vx0[:], xc0[:], xs0[:], AOP.is_equal)
            vx1 = tl(); TT(vx1[:], xc1[:], xs1[:], AOP.is_equal)
            gx = tl(); TS(gx[:], fx[:], -1.0, AOP.mult, 1.0, AOP.add)
            wx0 = tl(); TT(wx0[:], gx[:], vx0[:], AOP.mult)
            wx1 = tl(); TT(wx1[:], fx[:], vx1[:], AOP.mult)
            xst = tl(); TS(xst[:], xs0[:], 0.0, AOP.max, 62.0, AOP.min)
            e0 = tl(); TT(e0[:], xst[:], xs0[:], AOP.is_equal)
            em = tl(); TS(em[:], xs0[:], -1.0, AOP.is_equal)
            ep = tl(); TS(ep[:], xs0[:], 63.0, AOP.is_equal)
            l0a = tl(); TT(l0a[:], wx0[:], e0[:], AOP.mult)
            l0b = tl(); TT(l0b[:], wx1[:], em[:], AOP.mult)
            lane0 = tl(); TT(lane0[:], l0a[:], l0b[:], AOP.add)
            l1a = tl(); TT(l1a[:], wx1[:], e0[:], AOP.mult)
            l1b = tl(); TT(l1b[:], wx0[:], ep[:], AOP.mult)
            lane1 = tl(); TT(lane1[:], l1a[:], l1b[:], AOP.add)
            # final per-corner coefficients
            C00 = tl(); TT(C00[:], my0[:], lane0[:], AOP.mult)
            C01 = tl(); TT(C01[:], my0[:], lane1[:], AOP.mult)
            C10 = tl(); TT(C10[:], my1[:], lane0[:], AOP.mult)
            C11 = tl(); TT(C11[:], my1[:], lane1[:], AOP.mult)
            # gather row-pair base indices
            ib0 = tl()
            nc.vector.scalar_tensor_tensor(
                out=ib0[:], in0=yc0[:], scalar=64.0, in1=xst[:],
                op0=AOP.mult, op1=AOP.add,
            )
            ib1 = tl()
            nc.vector.scalar_tensor_tensor(
                out=ib1[:], in0=yc1[:], scalar=64.0, in1=xst[:],
                op0=AOP.mult, op1=AOP.add,
            )
            idx16 = cf.tile([128, 18, NPB], I16)
            nc.vector.tensor_copy(idx16[:, 0:9, :], ib0[:])
            nc.vector.tensor_copy(idx16[:, 9:18, :], ib1[:])

            if stage == "coef":
                dbg = nc.dram_tensor("dbg", [4, 128, K2, NPB], F32, kind="ExternalOutput")
                for ci, cpl in enumerate((C00, C01, C10, C11)):
                    nc.gpsimd.dma_start(dbg[:].rearrange("f p k b -> f p k b")[ci], cpl[:])
                dbg2 = nc.dram_tensor("dbg2", [128, 18, NPB], F32, kind="ExternalOutput")
                nc.gpsimd.dma_start(dbg2[:, 0:9, :], ib0[:])
                nc.gpsimd.dma_start(dbg2[:, 9:18, :], ib1[:])
            # ---- wrap indices for dma_gather ([16]-wrapped, replicated) ----
            idxw = cf.tile([128, 18, 256], I16)
            for g in range(8):
                nc.gpsimd.dma_start(
                    idxw[0:16, :, g:256:8], idx16[16 * g : 16 * (g + 1), :, :]
                )
            for g in range(1, 8):
                nc.gpsimd.dma_start(
                    idxw[16 * g : 16 * (g + 1), :, :], idxw[0:16, :, :]
                )

            # ---- main loop ----
            acc = cf.tile([128, NPB, COUT], F32)
            nc.gpsimd.memset(acc[:], 0.0)

            src_ap = AP(
                tensor=xrows_d[:].tensor, offset=0,
                ap=[[128, NROWS - 1], [1, 256]],
            )
            CPLANES = (C00, C01, C10, C11)
            if stage == "coef" or stage == "conv":
                krange = []
            elif stage == "gi":
                dbg5 = nc.dram_tensor("dbg5", [128, 18, 256], I16, kind="ExternalOutput")
                nc.gpsimd.dma_start(dbg5[:], idxw[:])
                krange = []
            elif stage == "gh":
                idxh_d = nc.dram_tensor("idxh", [128, 256], I16, kind="ExternalInput")
                idxh = cf.tile([128, 256], I16)
                nc.gpsimd.dma_start(idxh[:], idxh_d[:])
                gh = gp.tile([128, 2, P], BF16, tag="G")
                nc.gpsimd.dma_gather(
                    gh[:], src_ap, idxh[:], P, P,
                    elem_size=256, elem_step=128, transpose=True,
                    single_packet=False,
                )
                dbg6 = nc.dram_tensor("dbg6", [128, 2, P], BF16, kind="ExternalOutput")
                nc.gpsimd.dma_start(dbg6[:], gh[:])
                krange = []
            elif stage in ("k1", "g", "gmm"):
                krange = [0]
            else:
                krange = list(range(K2))
            for k in krange:
                g0 = gp.tile([128, 2, P], BF16, tag="G")
                g1 = gp.tile([128, 2, P], BF16, tag="G")
                nc.gpsimd.dma_gather(
                    g0[:], src_ap, idxw[:, k, :], P, P,
                    elem_size=256, elem_step=128, transpose=True,
                    single_packet=False,
                )
                nc.gpsimd.dma_gather(
                    g1[:], src_ap, idxw[:, 9 + k, :], P, P,
                    elem_size=256, elem_step=128, transpose=True,
                    single_packet=False,
                )
                if stage == "g":
                    dbg3 = nc.dram_tensor("dbg3", [128, 2, P], BF16, kind="ExternalOutput")
                    nc.gpsimd.dma_start(dbg3[:], g0[:])
                    continue
                for pb in range(NPB):
                    pz = psZ.tile([128, 512], F32)
                    for j, (gt, lane) in enumerate(
                        ((g0, 0), (g0, 1), (g1, 0), (g1, 1))
                    ):
                        nc.tensor.matmul(
                            pz[:, ts(j, 128)],
                            gt[:, lane, ts(pb, 128)],
                            wm[:, k, :],
                            start=True, stop=True,
                        )
                    z4 = z4p.tile([128, 512], F32, tag="z4")
                    nc.scalar.copy(z4[:], pz[:])
                    if stage == "gmm":
                        if pb == 0:
                            dbg4 = nc.dram_tensor("dbg4", [128, 512], F32, kind="ExternalOutput")
                            nc.gpsimd.dma_start(dbg4[:], z4[:])
                        continue
                    for j in range(4):
                        nc.vector.scalar_tensor_tensor(
                            out=acc[:, pb, :],
                            in0=z4[:, ts(j, 128)],
                            scalar=CPLANES[j][:, k, pb : pb + 1],
                            in1=acc[:, pb, :],
                            op0=AOP.mult, op1=AOP.add,
                        )

            nc.gpsimd.dma_start(
                out_d[:].rearrange("(pb part) o -> part pb o", part=128), acc[:]
            )

    nc.compile()
    return nc


def _host_prep(x, weight, offset_w, offset_b, mask_w, mask_b):
    x = np.asarray(x, np.float32)
    weight = np.asarray(weight, np.float32)
    offset_w = np.asarray(offset_w, np.float32)
    offset_b = np.asarray(offset_b, np.float32)
    mask_w = np.asarray(mask_w, np.float32)
    mask_b = np.asarray(mask_b, np.float32)

    wmain = np.ascontiguousarray(
        np.transpose(weight.reshape(COUT, C, K2), (2, 1, 0))
    ).astype(ml_dtypes.bfloat16)
    ow = offset_w.reshape(18, C, K2)
    w41 = np.zeros((41, C, K2), np.float32)
    w41[0:9] = ow[0::2]
    w41[9:18] = ow[1::2]
    w41[32:41] = mask_w.reshape(9, C, K2)
    woff = np.ascontiguousarray(np.transpose(w41, (2, 1, 0)))
    bias41 = np.zeros((41, 1), np.float32)
    bias41[0:9, 0] = offset_b[0::2]
    bias41[9:18, 0] = offset_b[1::2]
    bias41[32:41, 0] = mask_b

    ps = np.arange(P)
    ho = (ps // W).reshape(NPB, 128).T.astype(np.float32)
    wo_ = (ps % W).reshape(NPB, 128).T.astype(np.float32)
    byk = np.empty((128, K2, NPB), np.float32)
    bxk = np.empty((128, K2, NPB), np.float32)
    for k in range(K2):
        byk[:, k, :] = ho + (k // 3 - 1)
        bxk[:, k, :] = wo_ + (k % 3 - 1)

    shared = dict(wmain=wmain, woff=woff, bias41=bias41, byk=byk, bxk=bxk)

    in_maps = []
    for b in range(B):
        xpad = np.zeros((C, HP, HP), np.float32)
        xpad[:, 1 : H + 1, 1 : W + 1] = x[b]
        xrows = np.zeros((NROWS, C), ml_dtypes.bfloat16)
        xrows[:P] = x[b].transpose(1, 2, 0).reshape(P, C).astype(ml_dtypes.bfloat16)
        in_maps.append(
            dict(xpad=xpad.reshape(C, HP * HP), xrows=xrows, **shared)
        )
    return in_maps


def kernel(x, weight, offset_w, offset_b, mask_w, mask_b):
    if "nc" not in _CACHE:
        _CACHE["nc"] = _build()
    nc = _CACHE["nc"]
    in_maps = _host_prep(x, weight, offset_w, offset_b, mask_w, mask_b)
    res = run_bass_kernel_spmd(nc, in_maps, list(range(B)))
    _CACHE["last_result"] = res
    out = np.empty((B, COUT, H, W), np.float32)
    for b in range(B):
        out[b] = res.results[b]["out"].T.reshape(COUT, H, W)
    return out


# revision 14
# speedup vs baseline: 1.9568x; 1.9568x over previous
"""DCNv2 (modulated deformable conv 3x3) for Trainium2, 8 NeuronCores.

Sharding: pure data-parallel over batch B=8 -> core b computes batch b.

Per-core algorithm (batch b, C=Cout=128, H=W=64, P=H*W=4096):
  1. PE (fp32): offset/mask conv as 9 accumulated matmuls over a zero-padded
     channel-major x ([128, 66*66] SBUF), output [41, P] channel-major
     (channels: 0:9 y-offsets, 9:18 x-offsets, 32:41 mask - 32-aligned for
     the engines' base-partition restriction).  ACT applies bias (+ sigmoid
     for mask rows) during PSUM evacuation.
  2. PE transposes [41,128] chunks -> p-major planes [128(p), 41, 32(pb)].
  3. DVE: bilinear coefficient planes.  floor() via the fp32 round trick
     (x - 0.5 + 1.5*2^23) - 1.5*2^23 (ties resolve either way; bilinear
     interpolation is continuous so both splits give identical samples).
     Per kernel-point k one gather index  idx = ysel*64 + xsel  with
     ysel = clip(floor(py), 0, 62), xsel = clip(floor(px), 0, 62), and four
     per-corner coefficients  C[yl][xl] = mask * ylane_yl * xlane_xl  where
     the lane coefficients remap the fetched span (ysel..+1) x (xsel..+1)
     onto the true bilinear corners including border clip/zero semantics.
  4. GPSIMD dma_gather (transpose=True) over a host-packed bf16 table
     x2[p] = [x[p], x[p+1], x[p+64], x[p+65]] ([P, 512] in DRAM): each
     int16 index fetches 1KB = all four bilinear corners x 128 channels,
     landing transposed as four [c, p] planes.  One gather per k.
  5. PE (bf16): per (k, corner, p-block): Z^T[p,o] = G[c,p-blk].T @ W_k[c,o]
     (gathered block as the stationary operand) -> PSUM [128, 4x128].
  6. DVE accumulates acc[p, o] += coef_corner[p] * Z^T straight from PSUM
     via scalar_tensor_tensor (per-partition scalar = per-position coef).
  7. Output [P, 128] (p-major) DMAd out; host transposes to [Cout, H, W].
"""

import sys

sys.path.insert(0, "/opt/trn_rl_repo")

import numpy as np
import ml_dtypes

import concourse.bacc as bacc
import concourse.bass as bass
import concourse.mybir as mybir
import concourse.tile as tile
from concourse.ap import AP
from concourse.bass import ts
from concourse.bass_utils import run_bass_kernel_spmd
from concourse.library_config import mlp as mlp_lib
from concourse.masks import make_identity

F32 = mybir.dt.float32
BF16 = mybir.dt.bfloat16
I16 = mybir.dt.int16

B, C, H, W = 8, 128, 64, 64
COUT = 128
K2 = 9
P = H * W            # 4096
NPB = P // 128       # 32 p-blocks
HP = H + 2           # padded side
MAGIC = 12582912.0   # 1.5 * 2**23
AOP = mybir.AluOpType
AF = mybir.ActivationFunctionType

_CACHE = {}


def _build():
    nc = bacc.Bacc("TRN2", target_bir_lowering=False)

    xpad_d = nc.dram_tensor("xpad", [128, HP * HP], F32, kind="ExternalInput")
    x2_d = nc.dram_tensor("x2rows", [P, 512], BF16, kind="ExternalInput")
    wmain_d = nc.dram_tensor("wmain", [K2, 128, COUT], BF16, kind="ExternalInput")
    woff_d = nc.dram_tensor("woff", [K2, 128, 41], F32, kind="ExternalInput")
    bias_d = nc.dram_tensor("bias41", [41, 1], F32, kind="ExternalInput")
    byk_d = nc.dram_tensor("byk", [128, K2, NPB], F32, kind="ExternalInput")
    bxk_d = nc.dram_tensor("bxk", [128, K2, NPB], F32, kind="ExternalInput")
    out_d = nc.dram_tensor("out", [P, COUT], F32, kind="ExternalOutput")

    with tile.TileContext(nc) as tc:
        with (
            tc.tile_pool(name="const", bufs=1) as cp,
            tc.tile_pool(name="coef", bufs=1) as cf,
            tc.tile_pool(name="gp", bufs=2) as gp,
            tc.tile_pool(name="psO", bufs=2, space="PSUM") as psO,
            tc.tile_pool(name="psT", bufs=2, space="PSUM") as psT,
            tc.tile_pool(name="psZ", bufs=4, space="PSUM") as psZ,
        ):
            nc.gpsimd.load_library(mlp_lib)

            # ---- constant loads (SP-engine HWDGE queues, off gpsimd) ----
            xpad = cp.tile([128, HP * HP], F32)
            nc.sync.dma_start(xpad[:], xpad_d[:])
            wm = cp.tile([128, K2, COUT], BF16)
            nc.sync.dma_start(wm[:], wmain_d[:].rearrange("k c o -> c k o"))
            wo = cp.tile([128, K2, 41], F32)
            nc.sync.dma_start(wo[:], woff_d[:].rearrange("k c j -> c k j"))
            bias = cp.tile([41, 1], F32)
            nc.sync.dma_start(bias[:], bias_d[:])
            byk = cp.tile([128, K2, NPB], F32)
            nc.sync.dma_start(byk[:], byk_d[:])
            bxk = cp.tile([128, K2, NPB], F32)
            nc.sync.dma_start(bxk[:], bxk_d[:])
            ident = cp.tile([64, 64], F32)
            make_identity(nc, ident[:])

            # ---- offset/mask conv: [41, P] channel-major ----
            offs_cm = cf.tile([41, P], F32)
            nc.gpsimd.memset(offs_cm[:], 0.0)
            xv = xpad[:].rearrange("c (h w) -> c h w", h=HP)
            for ch in range(8):
                po = psO.tile([41, 512], F32)
                r0 = ch * 8
                for k in range(K2):
                    ki, kj = k // 3, k % 3
                    rhs = xv[:, r0 + ki : r0 + ki + 8, kj : kj + W]
                    nc.tensor.matmul(
                        po[:], wo[:, k, :], rhs,
                        start=(k == 0), stop=(k == K2 - 1),
                    )
                sl = slice(ch * 512, (ch + 1) * 512)
                nc.scalar.activation(
                    offs_cm[0:18, sl], po[0:18, :], AF.Identity,
                    bias=bias[0:18, :], scale=1.0,
                )
                nc.scalar.activation(
                    offs_cm[32:41, sl], po[32:41, :], AF.Sigmoid,
                    bias=bias[32:41, :], scale=1.0,
                )

            # ---- transpose to p-major [128, 41, 32] ----
            offs_pm = cf.tile([128, 41, NPB], F32)
            for t in range(NPB):
                pt = psT.tile([128, 41], F32)
                nc.tensor.transpose(
                    pt[:], offs_cm[:, ts(t, 128)], ident[:41, :41]
                )
                nc.vector.tensor_copy(offs_pm[:, :, t], pt[:])

            offy = offs_pm[:, 0:9, :]
            offx = offs_pm[:, 9:18, :]
            mask = offs_pm[:, 32:41, :]

            # ---- coefficient planes (DVE, [128, 9, 32] each) ----
            SH = [128, K2, NPB]
            _tln = [0]

            def tl():
                _tln[0] += 1
                return cf.tile(SH, F32, name=f"cftmp{_tln[0]}")

            def TS(out, in0, s1, op0, s2=None, op1=None):
                kw = {"op1": op1} if op1 is not None else {}
                nc.vector.tensor_scalar(
                    out=out, in0=in0, scalar1=s1, scalar2=s2, op0=op0, **kw
                )

            def TT(out, a, b, op):
                nc.vector.tensor_tensor(out=out, in0=a, in1=b, op=op)

            # y side: floor, fractions, validity
            t0 = tl(); TS(t0[:], offy, -0.5, AOP.add, MAGIC, AOP.add)
            iy = tl(); TS(iy[:], t0[:], MAGIC, AOP.subtract)
            fy = tl(); TT(fy[:], offy, iy[:], AOP.subtract)
            ys0 = tl(); TT(ys0[:], iy[:], byk[:], AOP.add)
            ys1 = tl(); TS(ys1[:], ys0[:], 1.0, AOP.add)
            yc0 = tl(); TS(yc0[:], ys0[:], 0.0, AOP.max, 63.0, AOP.min)
            yc1 = tl(); TS(yc1[:], ys1[:], 0.0, AOP.max, 63.0, AOP.min)
            vy0 = tl(); TT(vy0[:], yc0[:], ys0[:], AOP.is_equal)
            vy1 = tl(); TT(vy1[:], yc1[:], ys1[:], AOP.is_equal)
            gy = tl(); TS(gy[:], fy[:], -1.0, AOP.mult, 1.0, AOP.add)
            wy0 = tl(); TT(wy0[:], gy[:], vy0[:], AOP.mult)
            wy1 = tl(); TT(wy1[:], fy[:], vy1[:], AOP.mult)
            # y lane remap (fetched rows ysel, ysel+1)
            ysel = tl(); TS(ysel[:], ys0[:], 0.0, AOP.max, 62.0, AOP.min)
            f0 = tl(); TT(f0[:], ysel[:], ys0[:], AOP.is_equal)
            fm = tl(); TS(fm[:], ys0[:], -1.0, AOP.is_equal)
            fp = tl(); TS(fp[:], ys0[:], 63.0, AOP.is_equal)
            ya = tl(); TT(ya[:], wy0[:], f0[:], AOP.mult)
            yb = tl(); TT(yb[:], wy1[:], fm[:], AOP.mult)
            ylane0 = tl(); TT(ylane0[:], ya[:], yb[:], AOP.add)
            yc_ = tl(); TT(yc_[:], wy1[:], f0[:], AOP.mult)
            yd = tl(); TT(yd[:], wy0[:], fp[:], AOP.mult)
            ylane1 = tl(); TT(ylane1[:], yc_[:], yd[:], AOP.add)
            myl0 = tl(); TT(myl0[:], ylane0[:], mask, AOP.mult)
            myl1 = tl(); TT(myl1[:], ylane1[:], mask, AOP.mult)
            # x side
            t1 = tl(); TS(t1[:], offx, -0.5, AOP.add, MAGIC, AOP.add)
            ix = tl(); TS(ix[:], t1[:], MAGIC, AOP.subtract)
            fx = tl(); TT(fx[:], offx, ix[:], AOP.subtract)
            xs0 = tl(); TT(xs0[:], ix[:], bxk[:], AOP.add)
            xs1 = tl(); TS(xs1[:], xs0[:], 1.0, AOP.add)
            xc0 = tl(); TS(xc0[:], xs0[:], 0.0, AOP.max, 63.0, AOP.min)
            xc1 = tl(); TS(xc1[:], xs1[:], 0.0, AOP.max, 63.0, AOP.min)
            vx0 = tl(); TT(vx0[:], xc0[:], xs0[:], AOP.is_equal)
            vx1 = tl(); TT(vx1[:], xc1[:], xs1[:], AOP.is_equal)
            gx = tl(); TS(gx[:], fx[:], -1.0, AOP.mult, 1.0, AOP.add)
            wx0 = tl(); TT(wx0[:], gx[:], vx0[:], AOP.mult)
            wx1 = tl(); TT(wx1[:], fx[:], vx1[:], AOP.mult)
            xst = tl(); TS(xst[:], xs0[:], 0.0, AOP.max, 62.0, AOP.min)
            e0 = tl(); TT(e0[:], xst[:], xs0[:], AOP.is_equal)
            em = tl(); TS(em[:], xs0[:], -1.0, AOP.is_equal)
            ep = tl(); TS(ep[:], xs0[:], 63.0, AOP.is_equal)
            l0a = tl(); TT(l0a[:], wx0[:], e0[:], AOP.mult)
            l0b = tl(); TT(l0b[:], wx1[:], em[:], AOP.mult)
            xlane0 = tl(); TT(xlane0[:], l0a[:], l0b[:], AOP.add)
            l1a = tl(); TT(l1a[:], wx1[:], e0[:], AOP.mult)
            l1b = tl(); TT(l1b[:], wx0[:], ep[:], AOP.mult)
            xlane1 = tl(); TT(xlane1[:], l1a[:], l1b[:], AOP.add)
            # final per-corner coefficients (gather quarter order:
            # (y0,x0), (y0,x1), (y1,x0), (y1,x1))
            C00 = tl(); TT(C00[:], myl0[:], xlane0[:], AOP.mult)
            C01 = tl(); TT(C01[:], myl0[:], xlane1[:], AOP.mult)
            C10 = tl(); TT(C10[:], myl1[:], xlane0[:], AOP.mult)
            C11 = tl(); TT(C11[:], myl1[:], xlane1[:], AOP.mult)
            # gather base index = ysel*64 + xsel
            ib = tl()
            nc.vector.scalar_tensor_tensor(
                out=ib[:], in0=ysel[:], scalar=64.0, in1=xst[:],
                op0=AOP.mult, op1=AOP.add,
            )
            idx16 = cf.tile([128, K2, NPB], I16)
            nc.vector.tensor_copy(idx16[:], ib[:])

            # ---- wrap indices for dma_gather ([16]-wrapped, replicated) ----
            idxw = cf.tile([128, K2, 256], I16)
            for g in range(8):
                nc.sync.dma_start(
                    idxw[0:16, :, g:256:8], idx16[16 * g : 16 * (g + 1), :, :]
                )
            for g in range(1, 8):
                nc.sync.dma_start(
                    idxw[16 * g : 16 * (g + 1), :, :], idxw[0:16, :, :]
                )

            # ---- main loop ----
            acc = cf.tile([128, NPB, COUT], F32)
            nc.gpsimd.memset(acc[:], 0.0)

            src_ap = AP(
                tensor=x2_d[:].tensor, offset=0, ap=[[512, P], [1, 512]]
            )
            CPLANES = (C00, C01, C10, C11)
            NIDX_CHUNK = 512
            NCH = P // NIDX_CHUNK
            for k in range(K2):
                gt = gp.tile([128, NCH, 4, NIDX_CHUNK], BF16, tag="G")
                for c8 in range(NCH):
                    nc.gpsimd.dma_gather(
                        gt[:, c8, :, :],
                        src_ap,
                        idxw[:, k, c8 * 32 : (c8 + 1) * 32],
                        NIDX_CHUNK, NIDX_CHUNK,
                        elem_size=512, elem_step=512, transpose=True,
                    )
                for pb in range(NPB):
                    pz = psZ.tile([128, 512], F32)
                    c8, sub = pb // 4, pb % 4
                    for j in range(4):
                        nc.tensor.matmul(
                            pz[:, ts(j, 128)],
                            gt[:, c8, j, ts(sub, 128)],
                            wm[:, k, :],
                            start=True, stop=True,
                        )
                    for j in range(4):
                        nc.vector.scalar_tensor_tensor(
                            out=acc[:, pb, :],
                            in0=pz[:, ts(j, 128)],
                            scalar=CPLANES[j][:, k, pb : pb + 1],
                            in1=acc[:, pb, :],
                            op0=AOP.mult, op1=AOP.add,
                        )

            nc.sync.dma_start(
                out_d[:].rearrange("(pb part) o -> part pb o", part=128), acc[:]
            )

    nc.compile()
    return nc


def _host_prep(x, weight, offset_w, offset_b, mask_w, mask_b):
    x = np.asarray(x, np.float32)
    weight = np.asarray(weight, np.float32)
    offset_w = np.asarray(offset_w, np.float32)
    offset_b = np.asarray(offset_b, np.float32)
    mask_w = np.asarray(mask_w, np.float32)
    mask_b = np.asarray(mask_b, np.float32)

    wmain = np.ascontiguousarray(
        np.transpose(weight.reshape(COUT, C, K2), (2, 1, 0))
    ).astype(ml_dtypes.bfloat16)
    ow = offset_w.reshape(18, C, K2)
    w41 = np.zeros((41, C, K2), np.float32)
    w41[0:9] = ow[0::2]
    w41[9:18] = ow[1::2]
    w41[32:41] = mask_w.reshape(9, C, K2)
    woff = np.ascontiguousarray(np.transpose(w41, (2, 1, 0)))
    bias41 = np.zeros((41, 1), np.float32)
    bias41[0:9, 0] = offset_b[0::2]
    bias41[9:18, 0] = offset_b[1::2]
    bias41[32:41, 0] = mask_b

    ps = np.arange(P)
    ho = (ps // W).reshape(NPB, 128).T.astype(np.float32)
    wo_ = (ps % W).reshape(NPB, 128).T.astype(np.float32)
    byk = np.empty((128, K2, NPB), np.float32)
    bxk = np.empty((128, K2, NPB), np.float32)
    for k in range(K2):
        byk[:, k, :] = ho + (k // 3 - 1)
        bxk[:, k, :] = wo_ + (k % 3 - 1)

    shared = dict(wmain=wmain, woff=woff, bias41=bias41, byk=byk, bxk=bxk)

    in_maps = []
    for b in range(B):
        xpad = np.zeros((C, HP, HP), np.float32)
        xpad[:, 1 : H + 1, 1 : W + 1] = x[b]
        xr = np.zeros((P + 66, C), ml_dtypes.bfloat16)
        xr[:P] = x[b].transpose(1, 2, 0).reshape(P, C).astype(ml_dtypes.bfloat16)
        x2 = np.ascontiguousarray(
            np.concatenate(
                [xr[0:P], xr[1 : P + 1], xr[64 : P + 64], xr[65 : P + 65]],
                axis=1,
            )
        )
        in_maps.append(
            dict(xpad=xpad.reshape(C, HP * HP), x2rows=x2, **shared)
        )
    return in_maps


def kernel(x, weight, offset_w, offset_b, mask_w, mask_b):
    if "nc" not in _CACHE:
        _CACHE["nc"] = _build()
    nc = _CACHE["nc"]
    in_maps = _host_prep(x, weight, offset_w, offset_b, mask_w, mask_b)
    res = run_bass_kernel_spmd(nc, in_maps, list(range(B)))
    _CACHE["last_result"] = res
    out = np.empty((B, COUT, H, W), np.float32)
    for b in range(B):
        out[b] = res.results[b]["out"].T.reshape(COUT, H, W)
    return out


# revision 18
# speedup vs baseline: 2.1199x; 1.0833x over previous
"""DCNv2 (modulated deformable conv 3x3) for Trainium2, 8 NeuronCores.

Sharding: pure data-parallel over batch B=8 -> core b computes batch b.

Per-core algorithm (batch b, C=Cout=128, H=W=64, P=H*W=4096):
  1. PE (fp32): offset/mask conv as 9 accumulated matmuls over a zero-padded
     channel-major x ([128, 66*66] SBUF), output [41, P] channel-major
     (channels: 0:9 y-offsets, 9:18 x-offsets, 32:41 mask - 32-aligned for
     the engines' base-partition restriction).  ACT applies bias (+ sigmoid
     for mask rows) during PSUM evacuation.
  2. PE transposes [41,128] chunks -> p-major planes [128(p), 41, 32(pb)].
  3. DVE: bilinear coefficient planes.  floor() via the fp32 round trick
     (x - 0.5 + 1.5*2^23) - 1.5*2^23 (ties resolve either way; bilinear
     interpolation is continuous so both splits give identical samples).
     Per kernel-point k one gather index  idx = ysel*64 + xsel  with
     ysel = clip(floor(py), 0, 62), xsel = clip(floor(px), 0, 62), and four
     per-corner coefficients  C[yl][xl] = mask * ylane_yl * xlane_xl  where
     the lane coefficients remap the fetched span (ysel..+1) x (xsel..+1)
     onto the true bilinear corners including border clip/zero semantics.
  4. GPSIMD dma_gather (transpose=True) over a host-packed bf16 table
     x2[p] = [x[p], x[p+1], x[p+64], x[p+65]] ([P, 512] in DRAM): each
     int16 index fetches 1KB = all four bilinear corners x 128 channels,
     landing transposed as four [c, p] planes.  One gather per k.
  5. PE (bf16): per (k, corner, p-block): Z^T[p,o] = G[c,p-blk].T @ W_k[c,o]
     (gathered block as the stationary operand) -> PSUM [128, 4x128].
  6. DVE accumulates acc[p, o] += coef_corner[p] * Z^T straight from PSUM
     via scalar_tensor_tensor (per-partition scalar = per-position coef).
  7. Output [P, 128] (p-major) DMAd out; host transposes to [Cout, H, W].
"""

import sys

sys.path.insert(0, "/opt/trn_rl_repo")

import numpy as np
import ml_dtypes

import concourse.bacc as bacc
import concourse.bass as bass
import concourse.mybir as mybir
import concourse.tile as tile
from concourse.ap import AP
from concourse.bass import ts
from concourse.bass_utils import run_bass_kernel_spmd
from concourse.library_config import mlp as mlp_lib
from concourse.masks import make_identity

F32 = mybir.dt.float32
BF16 = mybir.dt.bfloat16
I16 = mybir.dt.int16

B, C, H, W = 8, 128, 64, 64
COUT = 128
K2 = 9
P = H * W            # 4096
NPB = P // 128       # 32 p-blocks
HP = H + 2           # padded side
MAGIC = 12582912.0   # 1.5 * 2**23
AOP = mybir.AluOpType
AF = mybir.ActivationFunctionType

_CACHE = {}


def _build():
    nc = bacc.Bacc("TRN2", target_bir_lowering=False, num_swdge_queues=4)

    xpad_d = nc.dram_tensor("xpad", [128, HP * HP], F32, kind="ExternalInput")
    x2_d = nc.dram_tensor("x2rows", [P, 512], BF16, kind="ExternalInput")
    wmain_d = nc.dram_tensor("wmain", [K2, 128, COUT], BF16, kind="ExternalInput")
    woff_d = nc.dram_tensor("woff", [K2, 128, 41], F32, kind="ExternalInput")
    bias_d = nc.dram_tensor("bias41", [41, 1], F32, kind="ExternalInput")
    byk_d = nc.dram_tensor("byk", [128, K2, NPB], F32, kind="ExternalInput")
    bxk_d = nc.dram_tensor("bxk", [128, K2, NPB], F32, kind="ExternalInput")
    out_d = nc.dram_tensor("out", [P, COUT], F32, kind="ExternalOutput")

    with tile.TileContext(nc) as tc:
        with (
            tc.tile_pool(name="const", bufs=1) as cp,
            tc.tile_pool(name="coef", bufs=1) as cf,
            tc.tile_pool(name="gp", bufs=2) as gp,
        ):
            nc.gpsimd.load_library(mlp_lib)

            # ---- constant loads (SP-engine HWDGE queues, off gpsimd) ----
            xpad = cp.tile([128, HP * HP], F32)
            nc.sync.dma_start(xpad[:], xpad_d[:])
            wm = cp.tile([128, K2, COUT], BF16)
            nc.sync.dma_start(wm[:], wmain_d[:].rearrange("k c o -> c k o"))
            wo = cp.tile([128, K2, 41], F32)
            nc.sync.dma_start(wo[:], woff_d[:].rearrange("k c j -> c k j"))
            bias = cp.tile([41, 1], F32)
            nc.sync.dma_start(bias[:], bias_d[:])
            byk = cp.tile([128, K2, NPB], F32)
            nc.sync.dma_start(byk[:], byk_d[:])
            bxk = cp.tile([128, K2, NPB], F32)
            nc.sync.dma_start(bxk[:], bxk_d[:])
            ident = cp.tile([64, 64], F32)
            make_identity(nc, ident[:])

            # ---- offset/mask conv: [41, P] channel-major ----
            from contextlib import ExitStack
            _es1 = ExitStack()
            psO_ctx = _es1.enter_context(tc.tile_pool(name="psO", bufs=2, space="PSUM"))
            psT_ctx = _es1.enter_context(tc.tile_pool(name="psT", bufs=2, space="PSUM"))
            offs_cm = cf.tile([41, P], F32)
            nc.gpsimd.memset(offs_cm[:], 0.0)
            xv = xpad[:].rearrange("c (h w) -> c h w", h=HP)
            for ch in range(8):
                po = psO_ctx.tile([41, 512], F32)
                r0 = ch * 8
                for k in range(K2):
                    ki, kj = k // 3, k % 3
                    rhs = xv[:, r0 + ki : r0 + ki + 8, kj : kj + W]
                    nc.tensor.matmul(
                        po[:], wo[:, k, :], rhs,
                        start=(k == 0), stop=(k == K2 - 1),
                    )
                sl = slice(ch * 512, (ch + 1) * 512)
                nc.scalar.activation(
                    offs_cm[0:18, sl], po[0:18, :], AF.Identity,
                    bias=bias[0:18, :], scale=1.0,
                )
                nc.scalar.activation(
                    offs_cm[32:41, sl], po[32:41, :], AF.Sigmoid,
                    bias=bias[32:41, :], scale=1.0,
                )

            # ---- transpose to p-major [128, 41, 32] ----
            offs_pm = cf.tile([128, 41, NPB], F32)
            for t in range(NPB):
                pt = psT_ctx.tile([128, 41], F32)
                nc.tensor.transpose(
                    pt[:], offs_cm[:, ts(t, 128)], ident[:41, :41]
                )
                nc.vector.tensor_copy(offs_pm[:, :, t], pt[:])

            offy = offs_pm[:, 0:9, :]
            offx = offs_pm[:, 9:18, :]
            mask = offs_pm[:, 32:41, :]

            # ---- coefficient planes (DVE, [128, 9, 32] each) ----
            SH = [128, K2, NPB]
            _tln = [0]

            def tl():
                _tln[0] += 1
                return cf.tile(SH, F32, name=f"cftmp{_tln[0]}")

            def TS(out, in0, s1, op0, s2=None, op1=None):
                kw = {"op1": op1} if op1 is not None else {}
                nc.vector.tensor_scalar(
                    out=out, in0=in0, scalar1=s1, scalar2=s2, op0=op0, **kw
                )

            def TT(out, a, b, op):
                nc.vector.tensor_tensor(out=out, in0=a, in1=b, op=op)

            # y side: floor, fractions, validity
            t0 = tl(); TS(t0[:], offy, -0.5, AOP.add, MAGIC, AOP.add)
            iy = tl(); TS(iy[:], t0[:], MAGIC, AOP.subtract)
            fy = tl(); TT(fy[:], offy, iy[:], AOP.subtract)
            ys0 = tl(); TT(ys0[:], iy[:], byk[:], AOP.add)
            ys1 = tl(); TS(ys1[:], ys0[:], 1.0, AOP.add)
            yc0 = tl(); TS(yc0[:], ys0[:], 0.0, AOP.max, 63.0, AOP.min)
            yc1 = tl(); TS(yc1[:], ys1[:], 0.0, AOP.max, 63.0, AOP.min)
            vy0 = tl(); TT(vy0[:], yc0[:], ys0[:], AOP.is_equal)
            vy1 = tl(); TT(vy1[:], yc1[:], ys1[:], AOP.is_equal)
            gy = tl(); TS(gy[:], fy[:], -1.0, AOP.mult, 1.0, AOP.add)
            wy0 = tl(); TT(wy0[:], gy[:], vy0[:], AOP.mult)
            wy1 = tl(); TT(wy1[:], fy[:], vy1[:], AOP.mult)
            # y lane remap (fetched rows ysel, ysel+1)
            ysel = tl(); TS(ysel[:], ys0[:], 0.0, AOP.max, 62.0, AOP.min)
            f0 = tl(); TT(f0[:], ysel[:], ys0[:], AOP.is_equal)
            fm = tl(); TS(fm[:], ys0[:], -1.0, AOP.is_equal)
            fp = tl(); TS(fp[:], ys0[:], 63.0, AOP.is_equal)
            ya = tl(); TT(ya[:], wy0[:], f0[:], AOP.mult)
            yb = tl(); TT(yb[:], wy1[:], fm[:], AOP.mult)
            ylane0 = tl(); TT(ylane0[:], ya[:], yb[:], AOP.add)
            yc_ = tl(); TT(yc_[:], wy1[:], f0[:], AOP.mult)
            yd = tl(); TT(yd[:], wy0[:], fp[:], AOP.mult)
            ylane1 = tl(); TT(ylane1[:], yc_[:], yd[:], AOP.add)
            myl0 = tl(); TT(myl0[:], ylane0[:], mask, AOP.mult)
            myl1 = tl(); TT(myl1[:], ylane1[:], mask, AOP.mult)
            # x side
            t1 = tl(); TS(t1[:], offx, -0.5, AOP.add, MAGIC, AOP.add)
            ix = tl(); TS(ix[:], t1[:], MAGIC, AOP.subtract)
            fx = tl(); TT(fx[:], offx, ix[:], AOP.subtract)
            xs0 = tl(); TT(xs0[:], ix[:], bxk[:], AOP.add)
            xs1 = tl(); TS(xs1[:], xs0[:], 1.0, AOP.add)
            xc0 = tl(); TS(xc0[:], xs0[:], 0.0, AOP.max, 63.0, AOP.min)
            xc1 = tl(); TS(xc1[:], xs1[:], 0.0, AOP.max, 63.0, AOP.min)
            vx0 = tl(); TT(vx0[:], xc0[:], xs0[:], AOP.is_equal)
            vx1 = tl(); TT(vx1[:], xc1[:], xs1[:], AOP.is_equal)
            gx = tl(); TS(gx[:], fx[:], -1.0, AOP.mult, 1.0, AOP.add)
            wx0 = tl(); TT(wx0[:], gx[:], vx0[:], AOP.mult)
            wx1 = tl(); TT(wx1[:], fx[:], vx1[:], AOP.mult)
            xst = tl(); TS(xst[:], xs0[:], 0.0, AOP.max, 62.0, AOP.min)
            e0 = tl(); TT(e0[:], xst[:], xs0[:], AOP.is_equal)
            em = tl(); TS(em[:], xs0[:], -1.0, AOP.is_equal)
            ep = tl(); TS(ep[:], xs0[:], 63.0, AOP.is_equal)
            l0a = tl(); TT(l0a[:], wx0[:], e0[:], AOP.mult)
            l0b = tl(); TT(l0b[:], wx1[:], em[:], AOP.mult)
            xlane0 = tl(); TT(xlane0[:], l0a[:], l0b[:], AOP.add)
            l1a = tl(); TT(l1a[:], wx1[:], e0[:], AOP.mult)
            l1b = tl(); TT(l1b[:], wx0[:], ep[:], AOP.mult)
            xlane1 = tl(); TT(xlane1[:], l1a[:], l1b[:], AOP.add)
            # final per-corner coefficients (gather quarter order:
            # (y0,x0), (y0,x1), (y1,x0), (y1,x1))
            C00 = tl(); TT(C00[:], myl0[:], xlane0[:], AOP.mult)
            C01 = tl(); TT(C01[:], myl0[:], xlane1[:], AOP.mult)
            C10 = tl(); TT(C10[:], myl1[:], xlane0[:], AOP.mult)
            C11 = tl(); TT(C11[:], myl1[:], xlane1[:], AOP.mult)
            # gather base index = ysel*64 + xsel
            ib = tl()
            nc.vector.scalar_tensor_tensor(
                out=ib[:], in0=ysel[:], scalar=64.0, in1=xst[:],
                op0=AOP.mult, op1=AOP.add,
            )
            idx16 = cf.tile([128, K2, NPB], I16)
            nc.vector.tensor_copy(idx16[:], ib[:])

            # ---- wrap indices for dma_gather ([16]-wrapped, replicated) ----
            idxw = cf.tile([128, K2, 256], I16)
            for g in range(8):
                nc.sync.dma_start(
                    idxw[0:16, :, g:256:8], idx16[16 * g : 16 * (g + 1), :, :]
                )
            for np2 in (16, 32, 64):
                nc.sync.dma_start(
                    idxw[np2 : 2 * np2, :, :], idxw[0:np2, :, :]
                )

            _es1.close()
            _es2 = ExitStack()
            psZ = _es2.enter_context(tc.tile_pool(name="psZ", bufs=6, space="PSUM"))

            # ---- main loop ----
            acc = cf.tile([128, NPB, COUT], F32)
            nc.gpsimd.memset(acc[:], 0.0)

            src_ap = AP(
                tensor=x2_d[:].tensor, offset=0, ap=[[512, P], [1, 512]]
            )
            CPLANES = (C00, C01, C10, C11)
            NIDX_CHUNK = 512
            NCH = P // NIDX_CHUNK
            for k in range(K2):
                gt = gp.tile([128, NCH, 4, NIDX_CHUNK], BF16, tag="G")
                for c8 in range(NCH):
                    nc.gpsimd.dma_gather(
                        gt[:, c8, :, :],
                        src_ap,
                        idxw[:, k, c8 * 32 : (c8 + 1) * 32],
                        NIDX_CHUNK, NIDX_CHUNK,
                        elem_size=512, elem_step=512, transpose=True,
                        queue_num=(k * NCH + c8) % 4,
                    )
                for pb in range(NPB):
                    pz = psZ.tile([128, 512], F32)
                    c8, sub = pb // 4, pb % 4
                    for j in range(4):
                        nc.tensor.matmul(
                            pz[:, ts(j, 128)],
                            gt[:, c8, j, ts(sub, 128)],
                            wm[:, k, :],
                            start=True, stop=True,
                        )
                    for j in range(4):
                        nc.vector.scalar_tensor_tensor(
                            out=acc[:, pb, :],
                            in0=pz[:, ts(j, 128)],
                            scalar=CPLANES[j][:, k, pb : pb + 1],
                            in1=acc[:, pb, :],
                            op0=AOP.mult, op1=AOP.add,
                        )

            nc.sync.dma_start(
                out_d[:].rearrange("(pb part) o -> part pb o", part=128), acc[:]
            )
            _es2.close()

    nc.compile()
    return nc


def _host_prep(x, weight, offset_w, offset_b, mask_w, mask_b):
    x = np.asarray(x, np.float32)
    weight = np.asarray(weight, np.float32)
    offset_w = np.asarray(offset_w, np.float32)
    offset_b = np.asarray(offset_b, np.float32)
    mask_w = np.asarray(mask_w, np.float32)
    mask_b = np.asarray(mask_b, np.float32)

    wmain = np.ascontiguousarray(
        np.transpose(weight.reshape(COUT, C, K2), (2, 1, 0))
    ).astype(ml_dtypes.bfloat16)
    ow = offset_w.reshape(18, C, K2)
    w41 = np.zeros((41, C, K2), np.float32)
    w41[0:9] = ow[0::2]
    w41[9:18] = ow[1::2]
    w41[32:41] = mask_w.reshape(9, C, K2)
    woff = np.ascontiguousarray(np.transpose(w41, (2, 1, 0)))
    bias41 = np.zeros((41, 1), np.float32)
    bias41[0:9, 0] = offset_b[0::2]
    bias41[9:18, 0] = offset_b[1::2]
    bias41[32:41, 0] = mask_b

    ps = np.arange(P)
    ho = (ps // W).reshape(NPB, 128).T.astype(np.float32)
    wo_ = (ps % W).reshape(NPB, 128).T.astype(np.float32)
    byk = np.empty((128, K2, NPB), np.float32)
    bxk = np.empty((128, K2, NPB), np.float32)
    for k in range(K2):
        byk[:, k, :] = ho + (k // 3 - 1)
        bxk[:, k, :] = wo_ + (k % 3 - 1)

    shared = dict(wmain=wmain, woff=woff, bias41=bias41, byk=byk, bxk=bxk)

    in_maps = []
    for b in range(B):
        xpad = np.zeros((C, HP, HP), np.float32)
        xpad[:, 1 : H + 1, 1 : W + 1] = x[b]
        xr = np.zeros((P + 66, C), ml_dtypes.bfloat16)
        xr[:P] = x[b].transpose(1, 2, 0).reshape(P, C).astype(ml_dtypes.bfloat16)
        x2 = np.ascontiguousarray(
            np.concatenate(
                [xr[0:P], xr[1 : P + 1], xr[64 : P + 64], xr[65 : P + 65]],
                axis=1,
            )
        )
        in_maps.append(
            dict(xpad=xpad.reshape(C, HP * HP), x2rows=x2, **shared)
        )
    return in_maps


def kernel(x, weight, offset_w, offset_b, mask_w, mask_b):
    if "nc" not in _CACHE:
        _CACHE["nc"] = _build()
    nc = _CACHE["nc"]
    in_maps = _host_prep(x, weight, offset_w, offset_b, mask_w, mask_b)
    res = run_bass_kernel_spmd(nc, in_maps, list(range(B)))
    _CACHE["last_result"] = res
    out = np.empty((B, COUT, H, W), np.float32)
    for b in range(B):
        out[b] = res.results[b]["out"].T.reshape(COUT, H, W)
    return out


# revision 19
# speedup vs baseline: 2.6305x; 1.2408x over previous
"""DCNv2 (modulated deformable conv 3x3) for Trainium2, 8 NeuronCores.

Sharding: pure data-parallel over batch B=8 -> core b computes batch b.

Per-core algorithm (batch b, C=Cout=128, H=W=64, P=H*W=4096):
  1. PE (fp32): offset/mask conv as 9 accumulated matmuls over a zero-padded
     channel-major x ([128, 66*66] SBUF), output [41, P] channel-major
     (channels: 0:9 y-offsets, 9:18 x-offsets, 32:41 mask - 32-aligned for
     the engines' base-partition restriction).  ACT applies bias (+ sigmoid
     for mask rows) during PSUM evacuation.
  2. PE transposes [41,128] chunks -> p-major planes [128(p), 41, 32(pb)].
  3. DVE: bilinear coefficient planes.  floor() via the fp32 round trick
     (x - 0.5 + 1.5*2^23) - 1.5*2^23 (ties resolve either way; bilinear
     interpolation is continuous so both splits give identical samples).
     Per kernel-point k one gather index  idx = ysel*64 + xsel  with
     ysel = clip(floor(py), 0, 62), xsel = clip(floor(px), 0, 62), and four
     per-corner coefficients  C[yl][xl] = mask * ylane_yl * xlane_xl  where
     the lane coefficients remap the fetched span (ysel..+1) x (xsel..+1)
     onto the true bilinear corners including border clip/zero semantics.
  4. GPSIMD dma_gather (transpose=True) over a host-packed bf16 table
     x2[p] = [x[p], x[p+1], x[p+64], x[p+65]] ([P, 512] in DRAM): each
     int16 index fetches 1KB = all four bilinear corners x 128 channels,
     landing transposed as four [c, p] planes.  One gather per k.
  5. PE (bf16): per (k, corner, p-block): Z^T[p,o] = G[c,p-blk].T @ W_k[c,o]
     (gathered block as the stationary operand) -> PSUM [128, 4x128].
  6. DVE accumulates acc[p, o] += coef_corner[p] * Z^T straight from PSUM
     via scalar_tensor_tensor (per-partition scalar = per-position coef).
  7. Output [P, 128] (p-major) DMAd out; host transposes to [Cout, H, W].
"""

import sys

sys.path.insert(0, "/opt/trn_rl_repo")

import numpy as np
import ml_dtypes

import concourse.bacc as bacc
import concourse.bass as bass
import concourse.mybir as mybir
import concourse.tile as tile
from concourse.ap import AP
from concourse.bass import ts
from concourse.bass_utils import run_bass_kernel_spmd
from concourse.library_config import mlp as mlp_lib
from concourse.masks import make_identity

F32 = mybir.dt.float32
BF16 = mybir.dt.bfloat16
I16 = mybir.dt.int16

B, C, H, W = 8, 128, 64, 64
COUT = 128
K2 = 9
P = H * W            # 4096
NPB = P // 128       # 32 p-blocks
HP = H + 2           # padded side
MAGIC = 12582912.0   # 1.5 * 2**23
AOP = mybir.AluOpType
AF = mybir.ActivationFunctionType

_CACHE = {}


def _build():
    nc = bacc.Bacc("TRN2", target_bir_lowering=False, num_swdge_queues=4)

    xpad_d = nc.dram_tensor("xpad", [128, HP * HP], F32, kind="ExternalInput")
    x2_d = nc.dram_tensor("x2rows", [P, 512], BF16, kind="ExternalInput")
    wmain_d = nc.dram_tensor("wmain", [K2, 128, COUT], BF16, kind="ExternalInput")
    woff_d = nc.dram_tensor("woff", [K2, 128, 41], F32, kind="ExternalInput")
    bias_d = nc.dram_tensor("bias41", [41, 1], F32, kind="ExternalInput")
    byk_d = nc.dram_tensor("byk", [128, K2, NPB], F32, kind="ExternalInput")
    bxk_d = nc.dram_tensor("bxk", [128, K2, NPB], F32, kind="ExternalInput")
    out_d = nc.dram_tensor("out", [P, COUT], F32, kind="ExternalOutput")

    with tile.TileContext(nc) as tc:
        with (
            tc.tile_pool(name="const", bufs=1) as cp,
            tc.tile_pool(name="coef", bufs=1) as cf,
            tc.tile_pool(name="gp", bufs=2) as gp,
        ):
            nc.gpsimd.load_library(mlp_lib)

            # ---- constant loads (SP-engine HWDGE queues, off gpsimd) ----
            xpad = cp.tile([128, HP * HP], F32)
            nc.sync.dma_start(xpad[:], xpad_d[:])
            wm = cp.tile([128, K2, COUT], BF16)
            nc.sync.dma_start(wm[:], wmain_d[:].rearrange("k c o -> c k o"))
            wo = cp.tile([128, K2, 41], F32)
            nc.sync.dma_start(wo[:], woff_d[:].rearrange("k c j -> c k j"))
            bias = cp.tile([41, 1], F32)
            nc.sync.dma_start(bias[:], bias_d[:])
            byk = cp.tile([128, K2, NPB], F32)
            nc.sync.dma_start(byk[:], byk_d[:])
            bxk = cp.tile([128, K2, NPB], F32)
            nc.sync.dma_start(bxk[:], bxk_d[:])
            ident = cp.tile([64, 64], F32)
            make_identity(nc, ident[:])

            # ---- offset/mask conv: [41, P] channel-major ----
            from contextlib import ExitStack
            _es1 = ExitStack()
            psO_ctx = _es1.enter_context(tc.tile_pool(name="psO", bufs=2, space="PSUM"))
            psT_ctx = _es1.enter_context(tc.tile_pool(name="psT", bufs=2, space="PSUM"))
            offs_cm = cf.tile([41, P], F32)
            nc.gpsimd.memset(offs_cm[:], 0.0)
            xv = xpad[:].rearrange("c (h w) -> c h w", h=HP)
            for ch in range(8):
                po = psO_ctx.tile([41, 512], F32)
                r0 = ch * 8
                for k in range(K2):
                    ki, kj = k // 3, k % 3
                    rhs = xv[:, r0 + ki : r0 + ki + 8, kj : kj + W]
                    nc.tensor.matmul(
                        po[:], wo[:, k, :], rhs,
                        start=(k == 0), stop=(k == K2 - 1),
                    )
                sl = slice(ch * 512, (ch + 1) * 512)
                nc.scalar.activation(
                    offs_cm[0:18, sl], po[0:18, :], AF.Identity,
                    bias=bias[0:18, :], scale=1.0,
                )
                nc.scalar.activation(
                    offs_cm[32:41, sl], po[32:41, :], AF.Sigmoid,
                    bias=bias[32:41, :], scale=1.0,
                )

            # ---- transpose to p-major [128, 41, 32] ----
            offs_pm = cf.tile([128, 41, NPB], F32)
            for t in range(NPB):
                pt = psT_ctx.tile([128, 41], F32)
                nc.tensor.transpose(
                    pt[:], offs_cm[:, ts(t, 128)], ident[:41, :41]
                )
                nc.vector.tensor_copy(offs_pm[:, :, t], pt[:])

            offy = offs_pm[:, 0:9, :]
            offx = offs_pm[:, 9:18, :]
            mask = offs_pm[:, 32:41, :]

            # ---- coefficient planes (DVE, [128, 9, 32] each) ----
            SH = [128, K2, NPB]
            _tln = [0]

            def tl():
                _tln[0] += 1
                return cf.tile(SH, F32, name=f"cftmp{_tln[0]}")

            def TS(out, in0, s1, op0, s2=None, op1=None):
                kw = {"op1": op1} if op1 is not None else {}
                nc.vector.tensor_scalar(
                    out=out, in0=in0, scalar1=s1, scalar2=s2, op0=op0, **kw
                )

            def TT(out, a, b, op):
                nc.vector.tensor_tensor(out=out, in0=a, in1=b, op=op)

            # y side: floor, fractions, validity
            t0 = tl(); TS(t0[:], offy, -0.5, AOP.add, MAGIC, AOP.add)
            iy = tl(); TS(iy[:], t0[:], MAGIC, AOP.subtract)
            fy = tl(); TT(fy[:], offy, iy[:], AOP.subtract)
            ys0 = tl(); TT(ys0[:], iy[:], byk[:], AOP.add)
            ys1 = tl(); TS(ys1[:], ys0[:], 1.0, AOP.add)
            yc0 = tl(); TS(yc0[:], ys0[:], 0.0, AOP.max, 63.0, AOP.min)
            yc1 = tl(); TS(yc1[:], ys1[:], 0.0, AOP.max, 63.0, AOP.min)
            vy0 = tl(); TT(vy0[:], yc0[:], ys0[:], AOP.is_equal)
            vy1 = tl(); TT(vy1[:], yc1[:], ys1[:], AOP.is_equal)
            gy = tl(); TS(gy[:], fy[:], -1.0, AOP.mult, 1.0, AOP.add)
            wy0 = tl(); TT(wy0[:], gy[:], vy0[:], AOP.mult)
            wy1 = tl(); TT(wy1[:], fy[:], vy1[:], AOP.mult)
            # y lane remap (fetched rows ysel, ysel+1)
            ysel = tl(); TS(ysel[:], ys0[:], 0.0, AOP.max, 62.0, AOP.min)
            f0 = tl(); TT(f0[:], ysel[:], ys0[:], AOP.is_equal)
            fm = tl(); TS(fm[:], ys0[:], -1.0, AOP.is_equal)
            fp = tl(); TS(fp[:], ys0[:], 63.0, AOP.is_equal)
            ya = tl(); TT(ya[:], wy0[:], f0[:], AOP.mult)
            yb = tl(); TT(yb[:], wy1[:], fm[:], AOP.mult)
            ylane0 = tl(); TT(ylane0[:], ya[:], yb[:], AOP.add)
            yc_ = tl(); TT(yc_[:], wy1[:], f0[:], AOP.mult)
            yd = tl(); TT(yd[:], wy0[:], fp[:], AOP.mult)
            ylane1 = tl(); TT(ylane1[:], yc_[:], yd[:], AOP.add)
            myl0 = tl(); TT(myl0[:], ylane0[:], mask, AOP.mult)
            myl1 = tl(); TT(myl1[:], ylane1[:], mask, AOP.mult)
            # x side
            t1 = tl(); TS(t1[:], offx, -0.5, AOP.add, MAGIC, AOP.add)
            ix = tl(); TS(ix[:], t1[:], MAGIC, AOP.subtract)
            fx = tl(); TT(fx[:], offx, ix[:], AOP.subtract)
            xs0 = tl(); TT(xs0[:], ix[:], bxk[:], AOP.add)
            xs1 = tl(); TS(xs1[:], xs0[:], 1.0, AOP.add)
            xc0 = tl(); TS(xc0[:], xs0[:], 0.0, AOP.max, 63.0, AOP.min)
            xc1 = tl(); TS(xc1[:], xs1[:], 0.0, AOP.max, 63.0, AOP.min)
            vx0 = tl(); TT(vx0[:], xc0[:], xs0[:], AOP.is_equal)
            vx1 = tl(); TT(vx1[:], xc1[:], xs1[:], AOP.is_equal)
            gx = tl(); TS(gx[:], fx[:], -1.0, AOP.mult, 1.0, AOP.add)
            wx0 = tl(); TT(wx0[:], gx[:], vx0[:], AOP.mult)
            wx1 = tl(); TT(wx1[:], fx[:], vx1[:], AOP.mult)
            xst = tl(); TS(xst[:], xs0[:], 0.0, AOP.max, 62.0, AOP.min)
            e0 = tl(); TT(e0[:], xst[:], xs0[:], AOP.is_equal)
            em = tl(); TS(em[:], xs0[:], -1.0, AOP.is_equal)
            ep = tl(); TS(ep[:], xs0[:], 63.0, AOP.is_equal)
            l0a = tl(); TT(l0a[:], wx0[:], e0[:], AOP.mult)
            l0b = tl(); TT(l0b[:], wx1[:], em[:], AOP.mult)
            xlane0 = tl(); TT(xlane0[:], l0a[:], l0b[:], AOP.add)
            l1a = tl(); TT(l1a[:], wx1[:], e0[:], AOP.mult)
            l1b = tl(); TT(l1b[:], wx0[:], ep[:], AOP.mult)
            xlane1 = tl(); TT(xlane1[:], l1a[:], l1b[:], AOP.add)
            # final per-corner coefficients (gather quarter order:
            # (y0,x0), (y0,x1), (y1,x0), (y1,x1))
            C00 = tl(); TT(C00[:], myl0[:], xlane0[:], AOP.mult)
            C01 = tl(); TT(C01[:], myl0[:], xlane1[:], AOP.mult)
            C10 = tl(); TT(C10[:], myl1[:], xlane0[:], AOP.mult)
            C11 = tl(); TT(C11[:], myl1[:], xlane1[:], AOP.mult)
            # gather base index = ysel*64 + xsel
            ib = tl()
            nc.vector.scalar_tensor_tensor(
                out=ib[:], in0=ysel[:], scalar=64.0, in1=xst[:],
                op0=AOP.mult, op1=AOP.add,
            )
            idx16 = cf.tile([128, K2, NPB], I16)
            nc.vector.tensor_copy(idx16[:], ib[:])

            # ---- wrap indices for dma_gather ([16]-wrapped, replicated) ----
            # per-k so k=0's chain completes first and gathers overlap
            idxw = cf.tile([128, K2, 256], I16)
            for k in range(K2):
                for g in range(8):
                    nc.sync.dma_start(
                        idxw[0:16, k, g:256:8],
                        idx16[16 * g : 16 * (g + 1), k, :],
                    )
                for np2 in (16, 32, 64):
                    nc.sync.dma_start(
                        idxw[np2 : 2 * np2, k, :], idxw[0:np2, k, :]
                    )

            _es1.close()
            _es2 = ExitStack()
            psZ = _es2.enter_context(tc.tile_pool(name="psZ", bufs=6, space="PSUM"))
            z2p = _es2.enter_context(tc.tile_pool(name="z2p", bufs=4))

            # ---- main loop ----
            acc = cf.tile([128, NPB, COUT], F32)
            nc.gpsimd.memset(acc[:], 0.0)

            src_ap = AP(
                tensor=x2_d[:].tensor, offset=0, ap=[[512, P], [1, 512]]
            )
            CPLANES = (C00, C01, C10, C11)
            NIDX_CHUNK = 512
            NCH = P // NIDX_CHUNK
            for k in range(K2):
                gt = gp.tile([128, NCH, 4, NIDX_CHUNK], BF16, tag="G")
                for c8 in range(NCH):
                    nc.gpsimd.dma_gather(
                        gt[:, c8, :, :],
                        src_ap,
                        idxw[:, k, c8 * 32 : (c8 + 1) * 32],
                        NIDX_CHUNK, NIDX_CHUNK,
                        elem_size=512, elem_step=512, transpose=True,
                        queue_num=(k * NCH + c8) % 4,
                    )
                for pb in range(NPB):
                    pz = psZ.tile([128, 512], F32)
                    c8, sub = pb // 4, pb % 4
                    for j in range(4):
                        nc.tensor.matmul(
                            pz[:, ts(j, 128)],
                            gt[:, c8, j, ts(sub, 128)],
                            wm[:, k, :],
                            start=True, stop=True,
                        )
                    # corners 0,1: ACT scaled-copy, DVE pair-add
                    z2 = z2p.tile([128, 2, 128], F32, tag="z2")
                    for j in range(2):
                        nc.scalar.activation(
                            z2[:, j, :], pz[:, ts(j, 128)], AF.Copy,
                            scale=CPLANES[j][:, k, pb : pb + 1],
                        )
                    zt = z2p.tile([128, 128], F32, tag="zt")
                    nc.vector.tensor_tensor(
                        out=zt[:], in0=z2[:, 0, :], in1=z2[:, 1, :], op=AOP.add
                    )
                    nc.vector.tensor_tensor(
                        out=acc[:, pb, :], in0=zt[:], in1=acc[:, pb, :],
                        op=AOP.add,
                    )
                    # corners 2,3: DVE fused scale-accumulate from PSUM
                    for j in range(2, 4):
                        nc.vector.scalar_tensor_tensor(
                            out=acc[:, pb, :],
                            in0=pz[:, ts(j, 128)],
                            scalar=CPLANES[j][:, k, pb : pb + 1],
                            in1=acc[:, pb, :],
                            op0=AOP.mult, op1=AOP.add,
                        )

            nc.sync.dma_start(
                out_d[:].rearrange("(pb part) o -> part pb o", part=128), acc[:]
            )
            _es2.close()

    nc.compile()
    return nc


def _host_prep(x, weight, offset_w, offset_b, mask_w, mask_b):
    x = np.asarray(x, np.float32)
    weight = np.asarray(weight, np.float32)
    offset_w = np.asarray(offset_w, np.float32)
    offset_b = np.asarray(offset_b, np.float32)
    mask_w = np.asarray(mask_w, np.float32)
    mask_b = np.asarray(mask_b, np.float32)

    wmain = np.ascontiguousarray(
        np.transpose(weight.reshape(COUT, C, K2), (2, 1, 0))
    ).astype(ml_dtypes.bfloat16)
    ow = offset_w.reshape(18, C, K2)
    w41 = np.zeros((41, C, K2), np.float32)
    w41[0:9] = ow[0::2]
    w41[9:18] = ow[1::2]
    w41[32:41] = mask_w.reshape(9, C, K2)
    woff = np.ascontiguousarray(np.transpose(w41, (2, 1, 0)))
    bias41 = np.zeros((41, 1), np.float32)
    bias41[0:9, 0] = offset_b[0::2]
    bias41[9:18, 0] = offset_b[1::2]
    bias41[32:41, 0] = mask_b

    ps = np.arange(P)
    ho = (ps // W).reshape(NPB, 128).T.astype(np.float32)
    wo_ = (ps % W).reshape(NPB, 128).T.astype(np.float32)
    byk = np.empty((128, K2, NPB), np.float32)
    bxk = np.empty((128, K2, NPB), np.float32)
    for k in range(K2):
        byk[:, k, :] = ho + (k // 3 - 1)
        bxk[:, k, :] = wo_ + (k % 3 - 1)

    shared = dict(wmain=wmain, woff=woff, bias41=bias41, byk=byk, bxk=bxk)

    in_maps = []
    for b in range(B):
        xpad = np.zeros((C, HP, HP), np.float32)
        xpad[:, 1 : H + 1, 1 : W + 1] = x[b]
        xr = np.zeros((P + 66, C), ml_dtypes.bfloat16)
        xr[:P] = x[b].transpose(1, 2, 0).reshape(P, C).astype(ml_dtypes.bfloat16)
        x2 = np.ascontiguousarray(
            np.concatenate(
                [xr[0:P], xr[1 : P + 1], xr[64 : P + 64], xr[65 : P + 65]],
                axis=1,
            )
        )
        in_maps.append(
            dict(xpad=xpad.reshape(C, HP * HP), x2rows=x2, **shared)
        )
    return in_maps


def kernel(x, weight, offset_w, offset_b, mask_w, mask_b):
    if "nc" not in _CACHE:
        _CACHE["nc"] = _build()
    nc = _CACHE["nc"]
    in_maps = _host_prep(x, weight, offset_w, offset_b, mask_w, mask_b)
    res = run_bass_kernel_spmd(nc, in_maps, list(range(B)))
    _CACHE["last_result"] = res
    out = np.empty((B, COUT, H, W), np.float32)
    for b in range(B):
        out[b] = res.results[b]["out"].T.reshape(COUT, H, W)
    return out


# revision 20
# speedup vs baseline: 2.6389x; 1.0032x over previous
"""DCNv2 (modulated deformable conv 3x3) for Trainium2, 8 NeuronCores.

Sharding: pure data-parallel over batch B=8 -> core b computes batch b.

Per-core algorithm (batch b, C=Cout=128, H=W=64, P=H*W=4096):
  1. PE (fp32): offset/mask conv as 9 accumulated matmuls over a zero-padded
     channel-major x ([128, 66*66] SBUF), output [41, P] channel-major
     (channels: 0:9 y-offsets, 9:18 x-offsets, 32:41 mask - 32-aligned for
     the engines' base-partition restriction).  ACT applies bias (+ sigmoid
     for mask rows) during PSUM evacuation.
  2. PE transposes [41,128] chunks -> p-major planes [128(p), 41, 32(pb)].
  3. DVE: bilinear coefficient planes.  floor() via the fp32 round trick
     (x - 0.5 + 1.5*2^23) - 1.5*2^23 (ties resolve either way; bilinear
     interpolation is continuous so both splits give identical samples).
     Per kernel-point k one gather index  idx = ysel*64 + xsel  with
     ysel = clip(floor(py), 0, 62), xsel = clip(floor(px), 0, 62), and four
     per-corner coefficients  C[yl][xl] = mask * ylane_yl * xlane_xl  where
     the lane coefficients remap the fetched span (ysel..+1) x (xsel..+1)
     onto the true bilinear corners including border clip/zero semantics.
  4. GPSIMD dma_gather (transpose=True) over a host-packed bf16 table
     x2[p] = [x[p], x[p+1], x[p+64], x[p+65]] ([P, 512] in DRAM): each
     int16 index fetches 1KB = all four bilinear corners x 128 channels,
     landing transposed as four [c, p] planes.  One gather per k.
  5. PE (bf16): per (k, corner, p-block): Z^T[p,o] = G[c,p-blk].T @ W_k[c,o]
     (gathered block as the stationary operand) -> PSUM [128, 4x128].
  6. DVE accumulates acc[p, o] += coef_corner[p] * Z^T straight from PSUM
     via scalar_tensor_tensor (per-partition scalar = per-position coef).
  7. Output [P, 128] (p-major) DMAd out; host transposes to [Cout, H, W].
"""

import sys

sys.path.insert(0, "/opt/trn_rl_repo")

import numpy as np
import ml_dtypes

import concourse.bacc as bacc
import concourse.bass as bass
import concourse.mybir as mybir
import concourse.tile as tile
from concourse.ap import AP
from concourse.bass import ts
from concourse.bass_utils import run_bass_kernel_spmd
from concourse.library_config import mlp as mlp_lib
from concourse.masks import make_identity

F32 = mybir.dt.float32
BF16 = mybir.dt.bfloat16
I16 = mybir.dt.int16

B, C, H, W = 8, 128, 64, 64
COUT = 128
K2 = 9
P = H * W            # 4096
NPB = P // 128       # 32 p-blocks
HP = H + 2           # padded side
MAGIC = 12582912.0   # 1.5 * 2**23
AOP = mybir.AluOpType
AF = mybir.ActivationFunctionType

_CACHE = {}


def _build():
    nc = bacc.Bacc("TRN2", target_bir_lowering=False, num_swdge_queues=4)

    xpad_d = nc.dram_tensor("xpad", [128, HP * HP], F32, kind="ExternalInput")
    x2_d = nc.dram_tensor("x2rows", [P, 512], BF16, kind="ExternalInput")
    wmain_d = nc.dram_tensor("wmain", [K2, 128, COUT], BF16, kind="ExternalInput")
    woff_d = nc.dram_tensor("woff", [K2, 128, 41], F32, kind="ExternalInput")
    bias_d = nc.dram_tensor("bias41", [41, 1], F32, kind="ExternalInput")
    byk_d = nc.dram_tensor("byk", [128, K2, NPB], F32, kind="ExternalInput")
    bxk_d = nc.dram_tensor("bxk", [128, K2, NPB], F32, kind="ExternalInput")
    out_d = nc.dram_tensor("out", [P, COUT], F32, kind="ExternalOutput")

    with tile.TileContext(nc) as tc:
        with (
            tc.tile_pool(name="const", bufs=1) as cp,
            tc.tile_pool(name="coef", bufs=1) as cf,
            tc.tile_pool(name="gp", bufs=2) as gp,
        ):
            nc.gpsimd.load_library(mlp_lib)

            # ---- constant loads (SP-engine HWDGE queues, off gpsimd) ----
            xpad = cp.tile([128, HP * HP], F32)
            nc.sync.dma_start(xpad[:], xpad_d[:])
            wm = cp.tile([128, K2, COUT], BF16)
            nc.sync.dma_start(wm[:], wmain_d[:].rearrange("k c o -> c k o"))
            wo = cp.tile([128, K2, 41], F32)
            nc.sync.dma_start(wo[:], woff_d[:].rearrange("k c j -> c k j"))
            bias = cp.tile([41, 1], F32)
            nc.sync.dma_start(bias[:], bias_d[:])
            byk = cp.tile([128, K2, NPB], F32)
            nc.sync.dma_start(byk[:], byk_d[:])
            bxk = cp.tile([128, K2, NPB], F32)
            nc.sync.dma_start(bxk[:], bxk_d[:])
            ident = cp.tile([64, 64], F32)
            make_identity(nc, ident[:])

            # ---- offset/mask conv: [41, P] channel-major ----
            from contextlib import ExitStack
            _es1 = ExitStack()
            psO_ctx = _es1.enter_context(tc.tile_pool(name="psO", bufs=2, space="PSUM"))
            psT_ctx = _es1.enter_context(tc.tile_pool(name="psT", bufs=2, space="PSUM"))
            offs_cm = cf.tile([41, P], F32)
            nc.gpsimd.memset(offs_cm[:], 0.0)
            xv = xpad[:].rearrange("c (h w) -> c h w", h=HP)
            for ch in range(8):
                po = psO_ctx.tile([41, 512], F32)
                r0 = ch * 8
                for k in range(K2):
                    ki, kj = k // 3, k % 3
                    rhs = xv[:, r0 + ki : r0 + ki + 8, kj : kj + W]
                    nc.tensor.matmul(
                        po[:], wo[:, k, :], rhs,
                        start=(k == 0), stop=(k == K2 - 1),
                    )
                sl = slice(ch * 512, (ch + 1) * 512)
                nc.scalar.activation(
                    offs_cm[0:18, sl], po[0:18, :], AF.Identity,
                    bias=bias[0:18, :], scale=1.0,
                )
                nc.scalar.activation(
                    offs_cm[32:41, sl], po[32:41, :], AF.Sigmoid,
                    bias=bias[32:41, :], scale=1.0,
                )

            # ---- transpose to p-major [128, 41, 32] ----
            offs_pm = cf.tile([128, 41, NPB], F32)
            for t in range(NPB):
                pt = psT_ctx.tile([128, 41], F32)
                nc.tensor.transpose(
                    pt[:], offs_cm[:, ts(t, 128)], ident[:41, :41]
                )
                nc.vector.tensor_copy(offs_pm[:, :, t], pt[:])

            offy = offs_pm[:, 0:9, :]
            offx = offs_pm[:, 9:18, :]
            mask = offs_pm[:, 32:41, :]

            # ---- coefficient planes (DVE, [128, 9, 32] each) ----
            SH = [128, K2, NPB]
            _tln = [0]

            def tl():
                _tln[0] += 1
                return cf.tile(SH, F32, name=f"cftmp{_tln[0]}")

            def TS(out, in0, s1, op0, s2=None, op1=None):
                kw = {"op1": op1} if op1 is not None else {}
                nc.vector.tensor_scalar(
                    out=out, in0=in0, scalar1=s1, scalar2=s2, op0=op0, **kw
                )

            def TT(out, a, b, op):
                nc.vector.tensor_tensor(out=out, in0=a, in1=b, op=op)

            # y side: floor, fractions, validity
            t0 = tl(); TS(t0[:], offy, -0.5, AOP.add, MAGIC, AOP.add)
            iy = tl(); TS(iy[:], t0[:], MAGIC, AOP.subtract)
            fy = tl(); TT(fy[:], offy, iy[:], AOP.subtract)
            ys0 = tl(); TT(ys0[:], iy[:], byk[:], AOP.add)
            ys1 = tl(); TS(ys1[:], ys0[:], 1.0, AOP.add)
            yc0 = tl(); TS(yc0[:], ys0[:], 0.0, AOP.max, 63.0, AOP.min)
            yc1 = tl(); TS(yc1[:], ys1[:], 0.0, AOP.max, 63.0, AOP.min)
            vy0 = tl(); TT(vy0[:], yc0[:], ys0[:], AOP.is_equal)
            vy1 = tl(); TT(vy1[:], yc1[:], ys1[:], AOP.is_equal)
            gy = tl(); TS(gy[:], fy[:], -1.0, AOP.mult, 1.0, AOP.add)
            wy0 = tl(); TT(wy0[:], gy[:], vy0[:], AOP.mult)
            wy1 = tl(); TT(wy1[:], fy[:], vy1[:], AOP.mult)
            # y lane remap (fetched rows ysel, ysel+1)
            ysel = tl(); TS(ysel[:], ys0[:], 0.0, AOP.max, 62.0, AOP.min)
            f0 = tl(); TT(f0[:], ysel[:], ys0[:], AOP.is_equal)
            fm = tl(); TS(fm[:], ys0[:], -1.0, AOP.is_equal)
            fp = tl(); TS(fp[:], ys0[:], 63.0, AOP.is_equal)
            ya = tl(); TT(ya[:], wy0[:], f0[:], AOP.mult)
            yb = tl(); TT(yb[:], wy1[:], fm[:], AOP.mult)
            ylane0 = tl(); TT(ylane0[:], ya[:], yb[:], AOP.add)
            yc_ = tl(); TT(yc_[:], wy1[:], f0[:], AOP.mult)
            yd = tl(); TT(yd[:], wy0[:], fp[:], AOP.mult)
            ylane1 = tl(); TT(ylane1[:], yc_[:], yd[:], AOP.add)
            myl0 = tl(); TT(myl0[:], ylane0[:], mask, AOP.mult)
            myl1 = tl(); TT(myl1[:], ylane1[:], mask, AOP.mult)
            # x side
            t1 = tl(); TS(t1[:], offx, -0.5, AOP.add, MAGIC, AOP.add)
            ix = tl(); TS(ix[:], t1[:], MAGIC, AOP.subtract)
            fx = tl(); TT(fx[:], offx, ix[:], AOP.subtract)
            xs0 = tl(); TT(xs0[:], ix[:], bxk[:], AOP.add)
            xs1 = tl(); TS(xs1[:], xs0[:], 1.0, AOP.add)
            xc0 = tl(); TS(xc0[:], xs0[:], 0.0, AOP.max, 63.0, AOP.min)
            xc1 = tl(); TS(xc1[:], xs1[:], 0.0, AOP.max, 63.0, AOP.min)
            vx0 = tl(); TT(vx0[:], xc0[:], xs0[:], AOP.is_equal)
            vx1 = tl(); TT(vx1[:], xc1[:], xs1[:], AOP.is_equal)
            gx = tl(); TS(gx[:], fx[:], -1.0, AOP.mult, 1.0, AOP.add)
            wx0 = tl(); TT(wx0[:], gx[:], vx0[:], AOP.mult)
            wx1 = tl(); TT(wx1[:], fx[:], vx1[:], AOP.mult)
            xst = tl(); TS(xst[:], xs0[:], 0.0, AOP.max, 62.0, AOP.min)
            e0 = tl(); TT(e0[:], xst[:], xs0[:], AOP.is_equal)
            em = tl(); TS(em[:], xs0[:], -1.0, AOP.is_equal)
            ep = tl(); TS(ep[:], xs0[:], 63.0, AOP.is_equal)
            l0a = tl(); TT(l0a[:], wx0[:], e0[:], AOP.mult)
            l0b = tl(); TT(l0b[:], wx1[:], em[:], AOP.mult)
            xlane0 = tl(); TT(xlane0[:], l0a[:], l0b[:], AOP.add)
            l1a = tl(); TT(l1a[:], wx1[:], e0[:], AOP.mult)
            l1b = tl(); TT(l1b[:], wx0[:], ep[:], AOP.mult)
            xlane1 = tl(); TT(xlane1[:], l1a[:], l1b[:], AOP.add)
            # final per-corner coefficients (gather quarter order:
            # (y0,x0), (y0,x1), (y1,x0), (y1,x1))
            C00 = tl(); TT(C00[:], myl0[:], xlane0[:], AOP.mult)
            C01 = tl(); TT(C01[:], myl0[:], xlane1[:], AOP.mult)
            C10 = tl(); TT(C10[:], myl1[:], xlane0[:], AOP.mult)
            C11 = tl(); TT(C11[:], myl1[:], xlane1[:], AOP.mult)
            # gather base index = ysel*64 + xsel
            ib = tl()
            nc.vector.scalar_tensor_tensor(
                out=ib[:], in0=ysel[:], scalar=64.0, in1=xst[:],
                op0=AOP.mult, op1=AOP.add,
            )
            idx16 = cf.tile([128, K2, NPB], I16)
            nc.vector.tensor_copy(idx16[:], ib[:])

            # ---- wrap indices for dma_gather ([16]-wrapped, replicated) ----
            # per-k so k=0's chain completes first and gathers overlap
            idxw = cf.tile([128, K2, 256], I16)
            for k in range(K2):
                for g in range(8):
                    nc.sync.dma_start(
                        idxw[0:16, k, g:256:8],
                        idx16[16 * g : 16 * (g + 1), k, :],
                    )
                for np2 in (16, 32, 64):
                    nc.sync.dma_start(
                        idxw[np2 : 2 * np2, k, :], idxw[0:np2, k, :]
                    )

            _es1.close()
            _es2 = ExitStack()
            psZ = _es2.enter_context(tc.tile_pool(name="psZ", bufs=6, space="PSUM"))
            z2p = _es2.enter_context(tc.tile_pool(name="z2p", bufs=4))

            # ---- main loop ----
            acc = cf.tile([128, NPB, COUT], F32)
            nc.gpsimd.memset(acc[:], 0.0)

            src_ap = AP(
                tensor=x2_d[:].tensor, offset=0, ap=[[512, P], [1, 512]]
            )
            CPLANES = (C00, C01, C10, C11)
            NIDX_CHUNK = 512
            NCH = P // NIDX_CHUNK
            for k in range(K2):
                gt = gp.tile([128, NCH, 4, NIDX_CHUNK], BF16, tag="G")
                for c8 in range(NCH):
                    nc.gpsimd.dma_gather(
                        gt[:, c8, :, :],
                        src_ap,
                        idxw[:, k, c8 * 32 : (c8 + 1) * 32],
                        NIDX_CHUNK, NIDX_CHUNK,
                        elem_size=512, elem_step=512, transpose=True,
                        queue_num=(k * NCH + c8) % 4,
                    )
                for pb in range(NPB):
                    pz = psZ.tile([128, 512], F32)
                    c8, sub = pb // 4, pb % 4
                    for j in range(4):
                        nc.tensor.matmul(
                            pz[:, ts(j, 128)],
                            gt[:, c8, j, ts(sub, 128)],
                            wm[:, k, :],
                            start=True, stop=True,
                        )
                    # corners 0,1: ACT scaled-copies; corners 2,3: DVE
                    # STTs that also fold in the ACT results; final TT pair
                    z2 = z2p.tile([128, 2, 128], F32, tag="z2")
                    for j in range(2):
                        nc.scalar.activation(
                            z2[:, j, :], pz[:, ts(j, 128)], AF.Copy,
                            scale=CPLANES[j][:, k, pb : pb + 1],
                        )
                    u = z2p.tile([128, 2, 128], F32, tag="u")
                    for j in (2, 3):
                        nc.vector.scalar_tensor_tensor(
                            out=u[:, j - 2, :],
                            in0=pz[:, ts(j, 128)],
                            scalar=CPLANES[j][:, k, pb : pb + 1],
                            in1=z2[:, j - 2, :],
                            op0=AOP.mult, op1=AOP.add,
                        )
                    zt = z2p.tile([128, 128], F32, tag="zt")
                    nc.vector.tensor_tensor(
                        out=zt[:], in0=u[:, 0, :], in1=u[:, 1, :], op=AOP.add
                    )
                    nc.vector.tensor_tensor(
                        out=acc[:, pb, :], in0=zt[:], in1=acc[:, pb, :],
                        op=AOP.add,
                    )

            nc.sync.dma_start(
                out_d[:].rearrange("(pb part) o -> part pb o", part=128), acc[:]
            )
            _es2.close()

    nc.compile()
    return nc


def _host_prep(x, weight, offset_w, offset_b, mask_w, mask_b):
    x = np.asarray(x, np.float32)
    weight = np.asarray(weight, np.float32)
    offset_w = np.asarray(offset_w, np.float32)
    offset_b = np.asarray(offset_b, np.float32)
    mask_w = np.asarray(mask_w, np.float32)
    mask_b = np.asarray(mask_b, np.float32)

    wmain = np.ascontiguousarray(
        np.transpose(weight.reshape(COUT, C, K2), (2, 1, 0))
    ).astype(ml_dtypes.bfloat16)
    ow = offset_w.reshape(18, C, K2)
    w41 = np.zeros((41, C, K2), np.float32)
    w41[0:9] = ow[0::2]
    w41[9:18] = ow[1::2]
    w41[32:41] = mask_w.reshape(9, C, K2)
    woff = np.ascontiguousarray(np.transpose(w41, (2, 1, 0)))
    bias41 = np.zeros((41, 1), np.float32)
    bias41[0:9, 0] = offset_b[0::2]
    bias41[9:18, 0] = offset_b[1::2]
    bias41[32:41, 0] = mask_b

    ps = np.arange(P)
    ho = (ps // W).reshape(NPB, 128).T.astype(np.float32)
    wo_ = (ps % W).reshape(NPB, 128).T.astype(np.float32)
    byk = np.empty((128, K2, NPB), np.float32)
    bxk = np.empty((128, K2, NPB), np.float32)
    for k in range(K2):
        byk[:, k, :] = ho + (k // 3 - 1)
        bxk[:, k, :] = wo_ + (k % 3 - 1)

    shared = dict(wmain=wmain, woff=woff, bias41=bias41, byk=byk, bxk=bxk)

    in_maps = []
    for b in range(B):
        xpad = np.zeros((C, HP, HP), np.float32)
        xpad[:, 1 : H + 1, 1 : W + 1] = x[b]
        xr = np.zeros((P + 66, C), ml_dtypes.bfloat16)
        xr[:P] = x[b].transpose(1, 2, 0).reshape(P, C).astype(ml_dtypes.bfloat16)
        x2 = np.ascontiguousarray(
            np.concatenate(
                [xr[0:P], xr[1 : P + 1], xr[64 : P + 64], xr[65 : P + 65]],
                axis=1,
            )
        )
        in_maps.append(
            dict(xpad=xpad.reshape(C, HP * HP), x2rows=x2, **shared)
        )
    return in_maps


def kernel(x, weight, offset_w, offset_b, mask_w, mask_b):
    if "nc" not in _CACHE:
        _CACHE["nc"] = _build()
    nc = _CACHE["nc"]
    in_maps = _host_prep(x, weight, offset_w, offset_b, mask_w, mask_b)
    res = run_bass_kernel_spmd(nc, in_maps, list(range(B)))
    _CACHE["last_result"] = res
    out = np.empty((B, COUT, H, W), np.float32)
    for b in range(B):
        out[b] = res.results[b]["out"].T.reshape(COUT, H, W)
    return out
